# revision 7
# baseline (speedup 1.0000x reference)
"""Distributed Trainium2 (8 NeuronCores) kernel for masked multi-head attention
+ output projection (nn_Attention_60790967107825).

v2: query compaction on top of the v1 key compaction.

The reference masks queries and keys with the same per-batch mask; masked
query rows collapse to uniform attention over ALL keys (a rank-1-per-batch
term).  v1 compacted keys only; v2 also compacts queries, so the whole
attention pipeline (scores, exp, PV, normalization, A2A payload, output
projection) runs on the ~50% surviving rows:

  - Each core owns 2 of 16 heads x 4 batches = 8 (b, h) pairs.  Per batch,
    the host keeps only unmasked rows (CLS + mask), in original order, so
    compacted queries split cleanly at c0_b into dest-core halves.
  - Scores are computed transposed (S^T = K Q^T, 64-deep contraction, no
    zero-padding memsets) in query chunks of <=1024 columns; the few
    overflow queries beyond 1024 pack all their key tiles into one PSUM
    tile so they cost a single exp instruction.
  - Z comes free from a ones-column appended to V; normalization is
    1/Z broadcast (DVE fast reciprocal -> DRAM round-trip partition
    broadcast in f32) fused into the PSUM evacuation multiply.
  - Two AllToAlls (~0.6MB/rank), one per head-half; #0 hides under the
    second half of attention, #1 is bridged by PE warmups pinned to the
    last fin DMA so they burn clock exactly during the exchange.
  - Each core projects its received compacted rows (<=640) against the
    full 1024x1024 W; masked rows are reconstructed on the host from a
    per-batch V-sum row carried through the collective and projected on
    device (pvm output).  Host scatters compacted rows back to their
    original positions and broadcast-fills masked rows.
"""

import os
import sys

import numpy as np

for _p in ("/opt/trn_rl_repo", "/root/.axon_site/_ro/trn_rl_repo"):
    if os.path.isdir(_p) and _p not in sys.path:
        sys.path.insert(0, _p)

import ml_dtypes  # noqa: E402
import concourse.bass as bass  # noqa: E402,F401
import concourse.mybir as mybir  # noqa: E402
import concourse.tile as tile  # noqa: E402
from concourse import bacc  # noqa: E402
from concourse.bass_utils import run_bass_kernel_spmd  # noqa: E402

B, H, N, D = 4, 16, 2048, 64
DIM = H * D
P = 128
NCORES = 8
HPC = H // NCORES          # heads per core
PAIRS = B * HPC            # (b, h_local) pairs per core
SCALE = float(D) ** -0.5
CT = DIM // P              # 8 contraction tiles in the projection
CHUNK = 1024               # query-chunk width (2 PSUM banks of f32)

bf16 = mybir.dt.bfloat16
f32 = mybir.dt.float32
npbf = ml_dtypes.bfloat16

_CACHE = {}


def _plan(c0s, c1s):
    """Derived sizes shared by graph builder and host prep."""
    Ms = tuple(a + b for a, b in zip(c0s, c1s))
    Mks = tuple(-(-m // P) * P for m in Ms)
    NQ = max(Ms)
    NK = max(Mks)
    CMAX = max(max(c0s), max(c1s))
    RP = -(-CMAX // P) * P          # projection rows per core (padded)
    VMCOL = -(-CMAX // 8) * 8       # v-sum column offset in a2a slots
    AW = VMCOL + 8                  # a2a slot width
    return Ms, Mks, NQ, NK, CMAX, RP, VMCOL, AW


def build_graph(c0s, c1s):
    Ms, Mks, NQ, NK, CMAX, RP, VMCOL, AW = _plan(c0s, c1s)
    JTK_MAX = max(Mks) // P
    nc = bacc.Bacc("TRN2", num_devices=NCORES)

    qT = nc.dram_tensor("qT", [PAIRS, D, NQ], bf16, kind="ExternalInput")
    kT = nc.dram_tensor("kT", [PAIRS, D, NK], bf16, kind="ExternalInput")
    vv = nc.dram_tensor("v", [PAIRS, NK, D + 1], bf16, kind="ExternalInput")
    wTD = nc.dram_tensor("wT", [DIM, DIM], bf16, kind="ExternalInput")
    boutD = nc.dram_tensor("bout", [1, DIM], f32, kind="ExternalInput")
    vmD = nc.dram_tensor("vmean", [PAIRS, D], bf16, kind="ExternalInput")
    outD = nc.dram_tensor("out", [RP, DIM], f32, kind="ExternalOutput")
    pvmD = nc.dram_tensor("pvm", [1, DIM], f32, kind="ExternalOutput")

    with tile.TileContext(nc, num_cores=NCORES) as tc:
        with tc.tile_pool(name="dram", bufs=1, space="DRAM") as dramp:
            a2a_in = [
                dramp.tile([NCORES, D, AW], bf16, name=f"a2a_in{h}")
                for h in range(HPC)
            ]
            a2a_out = [
                dramp.tile([NCORES, D, AW], bf16, name=f"a2a_out{h}")
                for h in range(HPC)
            ]
            zrow_dram = dramp.tile([PAIRS, NQ], f32, name="zrow_dram")

            with tc.tile_pool(name="constp", bufs=1) as constp:
                wt_sb = constp.tile([P, CT, DIM], bf16, name="wt_sb")
                bout128 = constp.tile([P, DIM], f32, name="bout128")
                gat = constp.tile([P, CT, RP + 8], bf16, name="gat")

                def prefetch_slice(step):
                    # spread const prefetch + vmean slot writes across the
                    # pair loop so no single burst clogs the sync queues;
                    # a2a_in[hl] vmean slots land before that half's trigger
                    if step < CT:
                        nc.sync.dma_start(
                            wt_sb[:, step, :], wTD[step * P : (step + 1) * P, :]
                        )
                    if step == 0:
                        nc.sync.dma_start(
                            bout128[:], boutD[0:1, :].to_broadcast((P, DIM))
                        )
                    if step < 4:
                        for vb in (2 * step, 2 * step + 1):
                            b4, vhl = vb % 4, vb // 4
                            vpr = b4 * HPC + vhl
                            for half in range(2):
                                nc.sync.dma_start(
                                    a2a_in[vhl][2 * b4 + half, :, VMCOL : VMCOL + 1],
                                    vmD[vpr : vpr + 1, :].rearrange("o d -> d o"),
                                )

                with (
                    tc.tile_pool(name="qkp", bufs=3) as qkp,
                    tc.tile_pool(name="vpool", bufs=3) as vp,
                    tc.tile_pool(name="ptp", bufs=3) as ptp,
                    tc.tile_pool(name="smallp", bufs=2) as smallp,
                    tc.tile_pool(name="finp", bufs=2) as finp,
                    tc.tile_pool(name="psS", bufs=2, space="PSUM") as psS,
                    tc.tile_pool(name="psO", bufs=2, space="PSUM") as psO,
                ):
                    first = True
                    last_fin_dma = None
                    pair_idx = 0
                    for hl in range(HPC):
                        for b in range(B):
                            pr = b * HPC + hl
                            M, Mk, c0 = Ms[b], Mks[b], c0s[b]
                            jtk = Mk // P
                            W1 = min(M, CHUNK)
                            W2 = M - W1
                            qt = qkp.tile([D, NQ], bf16, tag="qt", name=f"qt{pr}")
                            kt = qkp.tile([D, NK], bf16, tag="kt", name=f"kt{pr}")
                            # split loads: the first S matmul only needs the
                            # leading slices, so it can start sooner
                            ksplits = (
                                (0, P, 2 * P, 4 * P, Mk // 2, Mk)
                                if first
                                else (0, Mk)
                            )
                            for lo2, hi2 in zip(ksplits[:-1], ksplits[1:]):
                                if lo2 < hi2:
                                    nc.sync.dma_start(
                                        kt[:, lo2:hi2], kT[pr, :, lo2:hi2]
                                    )
                            qsplits = (
                                (0, 256, 512, W1, M) if first else (0, M)
                            )
                            for lo2, hi2 in zip(qsplits[:-1], qsplits[1:]):
                                if lo2 < hi2:
                                    nc.sync.dma_start(
                                        qt[:, lo2:hi2], qT[pr, :, lo2:hi2]
                                    )
                            vt = vp.tile(
                                [P, JTK_MAX, D + 1], bf16, tag="vt", name=f"vt{pr}"
                            )
                            vsplits = ((0, 1), (1, jtk)) if first else ((0, jtk),)
                            for lo, hi in vsplits:
                                if lo >= hi:
                                    continue
                                nc.sync.dma_start(
                                    vt[:, lo:hi, :],
                                    vv[pr, lo * P : hi * P, :]
                                    .rearrange("(t pp) d -> pp t d", pp=P),
                                )
                            if not first:
                                prefetch_slice(2 * (pair_idx - 1))
                                prefetch_slice(2 * (pair_idx - 1) + 1)
                            if first:
                                first = False
                            pair_idx += 1
                            if hl == 1 and b == 2:
                                # a2a#0 is long done: its gat loads dispatch
                                # with their semaphore already satisfied
                                for ct in range(CT):
                                    nc.sync.dma_start(
                                        gat[0:D, ct, 0:AW],
                                        a2a_out[0][ct],
                                    )

                            # ---- chunk 1: query columns [0, W1) ----
                            o1 = psO.tile(
                                [D + 1, CHUNK], f32, tag="ops", name=f"o1_{pr}"
                            )
                            for jt in range(jtk):
                                s_ps = psS.tile(
                                    [P, CHUNK], f32, tag="sps", name=f"s{pr}_{jt}"
                                )
                                for n0 in range(0, W1, 512):
                                    w = min(512, W1 - n0)
                                    nc.tensor.matmul(
                                        s_ps[:, n0 : n0 + w],
                                        lhsT=kt[:, jt * P : (jt + 1) * P],
                                        rhs=qt[:, n0 : n0 + w],
                                        start=True,
                                        stop=True,
                                    )
                                pt = ptp.tile(
                                    [P, CHUNK], bf16, tag="pt", name=f"p{pr}_{jt}"
                                )
                                nc.scalar.activation(
                                    pt[:, 0:W1],
                                    s_ps[:, 0:W1],
                                    mybir.ActivationFunctionType.Exp,
                                    scale=SCALE,
                                )
                                for n0 in range(0, W1, 512):
                                    w = min(512, W1 - n0)
                                    last_pv = nc.tensor.matmul(
                                        o1[:, n0 : n0 + w],
                                        lhsT=vt[:, jt, :],
                                        rhs=pt[:, n0 : n0 + w],
                                        start=(jt == 0),
                                        stop=(jt == jtk - 1),
                                    )
                            # z + evacuation for chunk 1 (overlaps chunk 2)
                            zp1 = smallp.tile(
                                [1, CHUNK], f32, tag="zp", name=f"zp1_{pr}"
                            )
                            nc.vector.tensor_copy(zp1[:, 0:W1], o1[D : D + 1, 0:W1])
                            zr1 = smallp.tile(
                                [1, CHUNK], f32, tag="zr", name=f"zr1_{pr}"
                            )
                            nc.vector.reciprocal_approx_fast(
                                zr1[:, 0:W1], zp1[:, 0:W1]
                            )
                            nc.gpsimd.dma_start(
                                zrow_dram[pr : pr + 1, 0:W1], zr1[:, 0:W1]
                            )
                            zm1 = finp.tile(
                                [D, CHUNK], f32, tag="zm", name=f"zm1_{pr}"
                            )
                            nc.gpsimd.dma_start(
                                zm1[:, 0:W1],
                                zrow_dram[pr : pr + 1, 0:W1].to_broadcast((D, W1)),
                            )
                            fin1 = finp.tile(
                                [D, CHUNK], bf16, tag="fin", name=f"fi1_{pr}"
                            )
                            last_fin = nc.vector.tensor_tensor(
                                fin1[:, 0:W1],
                                o1[0:D, 0:W1],
                                zm1[:, 0:W1],
                                mybir.AluOpType.mult,
                            )
                            nc.gpsimd.dma_start(
                                a2a_in[hl][2 * b, :, 0:c0], fin1[:, 0:c0]
                            )
                            last_fin_dma = nc.gpsimd.dma_start(
                                a2a_in[hl][2 * b + 1, :, 0 : W1 - c0],
                                fin1[:, c0:W1],
                            )

                            # ---- chunk 2: overflow queries [W1, M) ----
                            if W2 > 0:
                                s2 = psS.tile(
                                    [P, CHUNK], f32, tag="sps", name=f"s2_{pr}"
                                )
                                o2 = psS.tile(
                                    [P, CHUNK], f32, tag="sps", name=f"o2_{pr}"
                                )
                                for jt in range(jtk):
                                    nc.tensor.matmul(
                                        s2[:, jt * W2 : (jt + 1) * W2],
                                        lhsT=kt[:, jt * P : (jt + 1) * P],
                                        rhs=qt[:, W1:M],
                                        start=True,
                                        stop=True,
                                    )
                                pt2 = ptp.tile(
                                    [P, 256], bf16, tag="pt2", name=f"p2_{pr}"
                                )
                                nc.scalar.activation(
                                    pt2[:, 0 : jtk * W2],
                                    s2[:, 0 : jtk * W2],
                                    mybir.ActivationFunctionType.Exp,
                                    scale=SCALE,
                                )
                                for jt in range(jtk):
                                    last_pv = nc.tensor.matmul(
                                        o2[0 : D + 1, 0:W2],
                                        lhsT=vt[:, jt, :],
                                        rhs=pt2[:, jt * W2 : (jt + 1) * W2],
                                        start=(jt == 0),
                                        stop=(jt == jtk - 1),
                                    )
                                zp2 = smallp.tile(
                                    [1, CHUNK], f32, tag="zp", name=f"zp2_{pr}"
                                )
                                nc.vector.tensor_copy(
                                    zp2[:, 0:W2], o2[D : D + 1, 0:W2]
                                )
                                zr2 = smallp.tile(
                                    [1, CHUNK], f32, tag="zr", name=f"zr2_{pr}"
                                )
                                nc.vector.reciprocal_approx_fast(
                                    zr2[:, 0:W2], zp2[:, 0:W2]
                                )
                                nc.gpsimd.dma_start(
                                    zrow_dram[pr : pr + 1, W1:M], zr2[:, 0:W2]
                                )
                                zm2 = finp.tile(
                                    [D, 256], f32, tag="zm2", name=f"zm2_{pr}"
                                )
                                nc.gpsimd.dma_start(
                                    zm2[:, 0:W2],
                                    zrow_dram[pr : pr + 1, W1:M].to_broadcast(
                                        (D, W2)
                                    ),
                                )
                                fin2 = finp.tile(
                                    [D, 256], bf16, tag="fin2", name=f"fi2_{pr}"
                                )
                                last_fin = nc.vector.tensor_tensor(
                                    fin2[:, 0:W2],
                                    o2[0:D, 0:W2],
                                    zm2[:, 0:W2],
                                    mybir.AluOpType.mult,
                                )
                                last_fin_dma = nc.gpsimd.dma_start(
                                    a2a_in[hl][
                                        2 * b + 1, :, W1 - c0 : W1 - c0 + W2
                                    ],
                                    fin2[:, 0:W2],
                                )

                        # this head-half is complete on every core: exchange
                        # it (the hl=0 round is fully hidden under compute)
                        nc.gpsimd.collective_compute(
                            "AllToAll",
                            mybir.AluOpType.bypass,
                            replica_groups=[list(range(NCORES))],
                            ins=[a2a_in[hl].opt()],
                            outs=[a2a_out[hl].opt()],
                        )

                with (
                    tc.tile_pool(name="outp", bufs=3) as outp,
                    tc.tile_pool(name="smallq", bufs=1) as smallq,
                    tc.tile_pool(name="psP", bufs=2, space="PSUM") as psP,
                    tc.tile_pool(name="psPV", bufs=1, space="PSUM") as psPV,
                    tc.tile_pool(name="psWarm", bufs=1, space="PSUM") as psW,
                ):
                    for ct in range(CT):
                        nc.sync.dma_start(
                            gat[D : 2 * D, ct, 0:AW],
                            a2a_out[1][ct],
                        )

                    def pin(mm, after, why):
                        tile.add_dep_helper(mm.ins, after.ins, sync=False, reason=why)
                        return mm

                    # warmups pinned to the LAST a2a-in DMA: they start when
                    # the exchange starts and keep the PE clock at full HAM
                    # through the collective window
                    warm_ps = psW.tile([P, 512], f32, name="warm_ps")
                    last_warm = last_pv
                    NWARM, GRP = 56, 8
                    for wi in range(NWARM):
                        last_warm = pin(
                            nc.tensor.matmul(
                                warm_ps[:],
                                lhsT=wt_sb[:, 0, 0:128],
                                rhs=wt_sb[:, 1, 0:512],
                                start=(wi % GRP == 0),
                                stop=(wi % GRP == GRP - 1),
                            ),
                            last_fin_dma,
                            "warmups bridge the A2A window",
                        )

                    pvm_ps = psPV.tile([1, DIM], f32, name="pvm_ps")
                    for ct in range(CT):
                        for n0 in range(0, DIM, 512):
                            pin(
                                nc.tensor.matmul(
                                    pvm_ps[:, n0 : n0 + 512],
                                    lhsT=gat[:, ct, VMCOL : VMCOL + 1],
                                    rhs=wt_sb[:, ct, n0 : n0 + 512],
                                    start=(ct == 0),
                                    stop=(ct == CT - 1),
                                ),
                                last_warm,
                                "keep warmups ahead in the PE stream",
                            )
                    pvm_row = smallq.tile([1, DIM], f32, name="pvm_row")
                    pin(
                        nc.vector.tensor_copy(pvm_row[:], pvm_ps[:]),
                        last_fin,
                        "projection DVE ops stay behind attention DVE",
                    )
                    nc.sync.dma_start(pvmD[:], pvm_row[:])

                    for rt in range(RP // P):
                        o_ps = psP.tile([P, DIM], f32, tag="prps", name=f"pr{rt}")
                        for ct in range(CT):
                            for n0 in range(0, DIM, 512):
                                pin(
                                    nc.tensor.matmul(
                                        o_ps[:, n0 : n0 + 512],
                                        lhsT=gat[:, ct, rt * P : (rt + 1) * P],
                                        rhs=wt_sb[:, ct, n0 : n0 + 512],
                                        start=(ct == 0),
                                        stop=(ct == CT - 1),
                                    ),
                                    last_warm,
                                    "keep warmups ahead in the PE stream",
                                )
                        osb = outp.tile([P, DIM], f32, tag="osb", name=f"ob{rt}")
                        pin(
                            nc.vector.tensor_tensor(
                                osb[:], o_ps[:], bout128[:], mybir.AluOpType.add
                            ),
                            last_fin,
                            "projection DVE stays behind attention",
                        )
                        nc.sync.dma_start(outD[rt * P : (rt + 1) * P, :], osb[:])

    nc.compile()
    return nc


def _get_nc(c0s, c1s):
    key = (c0s, c1s)
    if key not in _CACHE:
        _CACHE[key] = build_graph(c0s, c1s)
    return _CACHE[key]


def mask_plan(mask):
    """Per-batch compacted-row indices and half counts."""
    m_full = np.concatenate(
        [np.ones((B, 1), dtype=bool), np.asarray(mask).astype(bool)], axis=1
    )  # [B, N]
    idxs = [np.flatnonzero(m_full[b]) for b in range(B)]
    c0s = tuple(int((i < N // 2).sum()) for i in idxs)
    c1s = tuple(len(i) - c for i, c in zip(idxs, c0s))
    return m_full, idxs, c0s, c1s


def make_in_maps(q, k, v, mask, W_out, b_out, idxs, c0s, c1s):
    Ms, Mks, NQ, NK, CMAX, RP, VMCOL, AW = _plan(c0s, c1s)
    q16 = np.asarray(q).astype(npbf)
    k16 = np.asarray(k).astype(npbf)
    v16 = np.asarray(v).astype(npbf)

    # compacted per batch: queries exact-width, keys padded to Mk with
    # zeros (zero keys score exp(0)=1 but carry 0 in the V ones-column,
    # so they add nothing to numerator or Z)
    qC = np.zeros((B, H, NQ, D), dtype=npbf)
    kC = np.zeros((B, H, NK, D), dtype=npbf)
    vC = np.zeros((B, H, NK, D + 1), dtype=npbf)
    for b in range(B):
        idx = idxs[b]
        qC[b, :, : len(idx)] = q16[b][:, idx, :]
        kC[b, :, : len(idx)] = k16[b][:, idx, :]
        vC[b, :, : len(idx), :D] = v16[b][:, idx, :]
        vC[b, :, : len(idx), D] = 1.0
    wT16 = np.ascontiguousarray(np.asarray(W_out).T).astype(npbf)
    bout = np.asarray(b_out).astype(np.float32).reshape(1, DIM)

    in_maps = []
    for c in range(NCORES):
        heads = slice(HPC * c, HPC * (c + 1))
        qTc = np.ascontiguousarray(
            qC[:, heads].transpose(0, 1, 3, 2).reshape(PAIRS, D, NQ)
        )
        kTc = np.ascontiguousarray(
            kC[:, heads].transpose(0, 1, 3, 2).reshape(PAIRS, D, NK)
        )
        vc = np.ascontiguousarray(vC[:, heads].reshape(PAIRS, NK, D + 1))
        vmc = (
            v16[:, heads].astype(np.float32).sum(axis=2).reshape(PAIRS, D)
        ).astype(npbf)
        in_maps.append(
            {
                "qT": qTc,
                "kT": kTc,
                "v": vc,
                "wT": wT16,
                "vmean": vmc,
                "bout": bout,
            }
        )
    return in_maps


def run(q, k, v, mask, W_out, b_out, trace=False, **spmd_kwargs):
    m_full, idxs, c0s, c1s = mask_plan(mask)
    nc = _get_nc(c0s, c1s)
    in_maps = make_in_maps(q, k, v, mask, W_out, b_out, idxs, c0s, c1s)
    res = run_bass_kernel_spmd(
        nc, in_maps, core_ids=list(range(NCORES)), trace=trace, **spmd_kwargs
    )
    bout = np.asarray(b_out).astype(np.float32).reshape(DIM)
    full = np.empty((B, N, DIM), dtype=np.float32)
    for b in range(B):
        r0 = np.asarray(res.results[2 * b]["out"])[: c0s[b]]
        r1 = np.asarray(res.results[2 * b + 1]["out"])[: c1s[b]]
        full[b, idxs[b]] = np.concatenate([r0, r1], axis=0)
        pvm = np.asarray(res.results[2 * b]["pvm"])[0]
        full[b, ~m_full[b]] = pvm * (1.0 / N) + bout
    return full, res


def kernel(q, k, v, mask, W_out, b_out):
    out, _ = run(q, k, v, mask, W_out, b_out, trace=False)
    return out
